# revision 1
# baseline (speedup 1.0000x reference)
"""Trainium2 Bass kernel for a dense transformer block (pre-norm, causal MHA + GELU FFN).

Distribution over 8 NeuronCores:
  Phase 1 (head tensor-parallel): every core holds the full normed activations
  (computed redundantly in transposed layout) and computes Q/K/V projections,
  causal attention and the per-head attention output for its 2 of 16 heads.
  One AllToAll per batch exchanges the thin [T, 256] attention-output slices so
  each core ends up with all 2048 head-dims for 1/8 of the tokens.
  Phase 2 (token-parallel): each core does out-projection + residual, rmsnorm
  and the full FFN for its 512 tokens, streaming the full FFN weights from HBM.

All matmuls run as float32r (full PE rate at free-dim >= 256, fp32 accumulate).
Activations stay fp32 end-to-end.
"""

import numpy as np
import ml_dtypes

# Model dims (hardcoded per the problem spec)
DIM = 2048
T = 2048
B = 2
H = 16
HD = 128
FF = 8192
EPS = 1e-5
SCALE = HD ** -0.5

NCORES = 8
P = 128
HPC = H // NCORES      # heads per core = 2
HDC = HPC * HD         # head dims per core = 256
DCH = DIM // P         # 16 chunks of the model dim
QB = 512               # query block
NQB = T // QB          # 4 query blocks per batch
ASH = T // NCORES      # tokens per A2A shard = 256
TPC = B * ASH          # tokens per core in phase 2 = 512
FCH = FF // P          # 64 ff chunks
FQ = 4                 # ff quarters
FPQ = FCH // FQ        # 16 ff chunks per quarter

_CACHE = {}
FFN_BF16 = False


def _build_program(reps=1, collectives=True):
    import concourse.mybir as mybir
    import concourse.tile as tile
    from concourse import bacc
    from concourse.masks import make_identity

    dt = mybir.dt
    f32 = dt.float32
    f32r = dt.float32r
    bf16 = dt.bfloat16
    ffdt = bf16 if FFN_BF16 else f32r
    htag = "bigh" if FFN_BF16 else "big"
    bbufs = 3 if FFN_BF16 else 4
    AF = mybir.ActivationFunctionType

    nc = bacc.Bacc("TRN2", target_bir_lowering=False, debug=False,
                   num_devices=NCORES)

    # ---- I/O ----
    xT_d = nc.dram_tensor("xT", [DIM, B * T], f32r, kind="ExternalInput")
    xres_d = nc.dram_tensor("xresT", [DIM, TPC], f32, kind="ExternalInput")
    wqT_d = nc.dram_tensor("wqT", [DIM, HDC], f32r, kind="ExternalInput")
    wkT_d = nc.dram_tensor("wkT", [DIM, HDC], f32r, kind="ExternalInput")
    wvT_d = nc.dram_tensor("wvT", [DIM, HDC], f32r, kind="ExternalInput")
    wo_d = nc.dram_tensor("wo_s", [DCH, P, DCH, P], f32r, kind="ExternalInput")
    w1_d = nc.dram_tensor("w1_s", [FCH, P, DCH, P], ffdt, kind="ExternalInput")
    w2_d = nc.dram_tensor("w2_s", [FQ, DCH, P, FPQ, P], ffdt,
                          kind="ExternalInput")
    mask_d = nc.dram_tensor("masks", [QB // P, P, QB], f32,
                            kind="ExternalInput")
    out_d = nc.dram_tensor("outT", [DIM, TPC], f32, kind="ExternalOutput")

    # ---- internal DRAM ----
    a2a_in = [nc.dram_tensor(f"a2a_in{b}", [T, HDC], f32) for b in range(B)]
    a2a_out = [nc.dram_tensor(f"a2a_out{b}", [NCORES, ASH, HDC], f32)
               for b in range(B)]
    x2_d = nc.dram_tensor("x2_save", [P, DCH, TPC], f32)

    xT_r = xT_d.ap().rearrange("(k p) t -> p k t", p=P)
    xres_r = xres_d.ap().rearrange("(k p) t -> p k t", p=P)
    out_r = out_d.ap().rearrange("(k p) t -> p k t", p=P)

    with tile.TileContext(nc) as tc:
        from contextlib import ExitStack
        with ExitStack() as ctx:
            consts = ctx.enter_context(tc.tile_pool(name="consts", bufs=1))
            ones = consts.tile([P, P], f32)
            nc.vector.memset(ones, 1.0)
            ones_r = consts.tile([P, P], f32r)
            nc.vector.tensor_copy(ones_r, ones)
            ident = consts.tile([P, P], f32)
            make_identity(nc, ident)

            for _rep in range(reps):
                # ============ PHASE 1 ============
                with ExitStack() as p1:
                    qkvw = p1.enter_context(tc.tile_pool(name="qkvw", bufs=3))
                    wq_sb = qkvw.tile([P, DCH, HDC], f32r, tag="w")
                    wk_sb = qkvw.tile([P, DCH, HDC], f32r, tag="w")
                    wv_sb = qkvw.tile([P, DCH, HDC], f32r, tag="w")
                    nc.sync.dma_start(wq_sb, wqT_d.ap().rearrange(
                        "(k p) n -> p k n", p=P))
                    nc.sync.dma_start(wk_sb, wkT_d.ap().rearrange(
                        "(k p) n -> p k n", p=P))
                    nc.sync.dma_start(wv_sb, wvT_d.ap().rearrange(
                        "(k p) n -> p k n", p=P))
                    mpool = p1.enter_context(tc.tile_pool(name="masks", bufs=1))
                    mask_sb = mpool.tile([P, QB // P, QB], f32)
                    nc.sync.dma_start(mask_sb, mask_d.ap().rearrange(
                        "r p q -> p r q"))

                    xb_pool = p1.enter_context(tc.tile_pool(name="xb", bufs=1))
                    sm_pool = p1.enter_context(tc.tile_pool(name="p1sm", bufs=2))
                    qkv_out = p1.enter_context(tc.tile_pool(name="qkvo", bufs=1))
                    q_pool = p1.enter_context(tc.tile_pool(name="qp", bufs=2))
                    exp_pool = p1.enter_context(tc.tile_pool(name="expp", bufs=3))
                    o_pool = p1.enter_context(tc.tile_pool(name="op", bufs=2))
                    on_pool = p1.enter_context(tc.tile_pool(name="onp", bufs=4))

                    ps1 = p1.enter_context(
                        tc.tile_pool(name="ps1", bufs=1, space="PSUM"))

                    for b in range(B):
                        kT = qkv_out.tile([P, HPC, T], f32r, tag="kT")
                        vn = qkv_out.tile([P, T // P, HDC], f32r, tag="vn")

                        for blk in range(NQB):
                            tok0 = b * T + blk * QB
                            xb = xb_pool.tile([P, DCH, QB], f32r, tag="xb")
                            nc.sync.dma_start(
                                xb, xT_r[:, :, tok0:tok0 + QB])
                            # sum of squares over the model dim (partition dim,
                            # 16 chunks) -> rms scale
                            acc = sm_pool.tile([P, QB], f32, tag="acc")
                            nc.vector.tensor_mul(acc, xb[:, 0, :], xb[:, 0, :])
                            for k in range(1, DCH):
                                sq = sm_pool.tile([P, QB], f32, tag="sq")
                                nc.vector.tensor_mul(sq, xb[:, k, :], xb[:, k, :])
                                nc.vector.tensor_add(acc, acc, sq)
                            ps_ss = ps1.tile([P, QB], f32, tag="psqk", bufs=2)
                            nc.tensor.matmul(ps_ss, ones, acc,
                                             start=True, stop=True)
                            ms = sm_pool.tile([P, QB], f32, tag="ms")
                            nc.vector.tensor_scalar(
                                ms, ps_ss, 1.0 / DIM, EPS,
                                mybir.AluOpType.mult, mybir.AluOpType.add)
                            rms = sm_pool.tile([P, QB], f32, tag="rms")
                            nc.scalar.activation(rms, ms, AF.Sqrt)
                            rsc = sm_pool.tile([P, QB], f32, tag="rsc")
                            nc.vector.reciprocal(rsc, rms)
                            # normalize in place (norm weight folded into W host-side)
                            for k in range(DCH):
                                nc.vector.tensor_mul(xb[:, k, :], xb[:, k, :], rsc)

                            # Q^T, K^T for this block: [hd 128, tok 512]
                            qloc = q_pool.tile([P, HPC, QB], f32r, tag="qloc")
                            for m in range(HPC):
                                ps = ps1.tile([P, QB], f32, tag="psqk", bufs=2)
                                for k in range(DCH):
                                    nc.tensor.matmul(
                                        ps,
                                        wq_sb[:, k, m * P:(m + 1) * P],
                                        xb[:, k, :],
                                        start=(k == 0), stop=(k == DCH - 1))
                                nc.vector.tensor_copy(qloc[:, m, :], ps)
                            for m in range(HPC):
                                ps = ps1.tile([P, QB], f32, tag="psqk", bufs=2)
                                for k in range(DCH):
                                    nc.tensor.matmul(
                                        ps,
                                        wk_sb[:, k, m * P:(m + 1) * P],
                                        xb[:, k, :],
                                        start=(k == 0), stop=(k == DCH - 1))
                                nc.vector.tensor_copy(
                                    kT[:, m, blk * QB:(blk + 1) * QB], ps)
                            # V natural: [tok 128, hd 256]
                            for ts in range(QB // P):
                                psf = ps1.tile([P, QB], f32, tag="psqk",
                                               bufs=2, name="psv")
                                ps = psf[:, :HDC]
                                for k in range(DCH):
                                    nc.tensor.matmul(
                                        ps,
                                        xb[:, k, ts * P:(ts + 1) * P],
                                        wv_sb[:, k, :],
                                        start=(k == 0), stop=(k == DCH - 1))
                                nc.vector.tensor_copy(vn[:, blk * 4 + ts, :], ps)

                            # ---- attention for q-block = blk (causal: only
                            # needs K/V blocks <= blk, all computed) ----
                            qb = blk
                            nkc = (qb + 1) * (QB // P)
                            for h in range(HPC):
                                psd = ps1.tile([P, QB], f32, tag="psden", bufs=1)
                                pso = ps1.tile([P, QB], f32, tag="pso", bufs=1)
                                for kc in range(nkc):
                                    psl = ps1.tile([P, QB], f32, tag="psl", bufs=3)
                                    nc.tensor.matmul(
                                        psl,
                                        kT[:, h, kc * P:(kc + 1) * P],
                                        qloc[:, h, :],
                                        start=True, stop=True)
                                    et = exp_pool.tile([P, QB], f32r, tag="et")
                                    nc.scalar.activation(et, psl, AF.Exp,
                                                         scale=SCALE)
                                    rel = kc - qb * (QB // P)
                                    if rel >= 0:
                                        nc.vector.tensor_mul(
                                            et, et, mask_sb[:, rel, :])
                                    nc.tensor.matmul(psd, ones_r, et,
                                                     start=(kc == 0),
                                                     stop=(kc == nkc - 1))
                                    nc.tensor.matmul(
                                        pso,
                                        vn[:, kc, h * P:(h + 1) * P],
                                        et,
                                        start=(kc == 0), stop=(kc == nkc - 1))
                                rden = sm_pool.tile([P, QB], f32, tag="rden")
                                nc.vector.reciprocal(rden, psd)
                                osb = o_pool.tile([P, QB], f32, tag="osb")
                                nc.vector.tensor_mul(osb, pso, rden)
                                for i in range(QB // P):
                                    pst = ps1.tile([P, P], f32, tag="pstr", bufs=1)
                                    nc.tensor.transpose(
                                        pst, osb[:, i * P:(i + 1) * P], ident)
                                    on = on_pool.tile([P, P], f32, tag="on")
                                    nc.vector.tensor_copy(on, pst)
                                    t0 = qb * QB + i * P
                                    nc.sync.dma_start(
                                        a2a_in[b].ap()[t0:t0 + P,
                                                       h * P:(h + 1) * P],
                                        on)

                        if collectives:
                            nc.gpsimd.collective_compute(
                                "AllToAll",
                                mybir.AluOpType.bypass,
                                replica_groups=[list(range(NCORES))],
                                ins=[a2a_in[b].ap()],
                                outs=[a2a_out[b].ap()],
                            )

                # ============ PHASE 2 ============
                with ExitStack() as p2:
                    big = p2.enter_context(tc.tile_pool(name="p2big", bufs=4))
                    seg_pool = p2.enter_context(tc.tile_pool(name="segp", bufs=3))
                    sm2 = p2.enter_context(tc.tile_pool(name="p2sm", bufs=1))
                    wstream = p2.enter_context(tc.tile_pool(name="wstr", bufs=3))
                    w2stream = p2.enter_context(tc.tile_pool(name="w2str", bufs=2))
                    ps2 = p2.enter_context(
                        tc.tile_pool(name="ps2", bufs=1, space="PSUM"))

                    # gather + transpose a2a segments into oT [hd-chunk, tok]
                    oT = big.tile([P, DCH, TPC], f32r, tag="big", bufs=bbufs)
                    for b in range(B):
                        for i in range(NCORES):
                            seg = seg_pool.tile([P, ASH // P, HDC], f32,
                                                tag="seg")
                            nc.sync.dma_start(
                                seg,
                                a2a_out[b].ap()[i].rearrange(
                                    "(s p) h -> p s h", p=P))
                            for ts in range(ASH // P):
                                for hs in range(HPC):
                                    pst = ps2.tile([P, P], f32, tag="pst2", bufs=2)
                                    nc.tensor.transpose(
                                        pst, seg[:, ts, hs * P:(hs + 1) * P],
                                        ident)
                                    nc.vector.tensor_copy(
                                        oT[:, i * HPC + hs,
                                           b * ASH + ts * P:
                                           b * ASH + (ts + 1) * P],
                                        pst)

                    # out-projection + residual -> x2T
                    x2T = big.tile([P, DCH, TPC], f32, tag="big", bufs=bbufs)
                    for m in range(DCH):
                        wo_sb = wstream.tile([P, DCH, P], f32r, tag="wmat", bufs=2)
                        nc.sync.dma_start(wo_sb, wo_d.ap()[m])
                        ps = ps2.tile([P, TPC], f32, tag="ps2w", bufs=3)
                        for kc in range(DCH):
                            nc.tensor.matmul(ps, wo_sb[:, kc, :],
                                             oT[:, kc, :],
                                             start=(kc == 0),
                                             stop=(kc == DCH - 1))
                        xres_c = sm2.tile([P, TPC], f32, tag="xresc", bufs=2)
                        nc.sync.dma_start(xres_c, xres_r[:, m, :])
                        nc.vector.tensor_add(x2T[:, m, :], ps, xres_c)
                    nc.sync.dma_start(x2_d.ap(), x2T)

                    # rmsnorm -> hT
                    acc = sm2.tile([P, TPC], f32, tag="acc2")
                    nc.vector.tensor_mul(acc, x2T[:, 0, :], x2T[:, 0, :])
                    for k in range(1, DCH):
                        sq = sm2.tile([P, TPC], f32, tag="sq2", bufs=2)
                        nc.vector.tensor_mul(sq, x2T[:, k, :], x2T[:, k, :])
                        nc.vector.tensor_add(acc, acc, sq)
                    ps_ss = ps2.tile([P, TPC], f32, tag="ps2w", bufs=3)
                    nc.tensor.matmul(ps_ss, ones, acc, start=True, stop=True)
                    ms2 = sm2.tile([P, TPC], f32, tag="ms2")
                    nc.vector.tensor_scalar(
                        ms2, ps_ss, 1.0 / DIM, EPS,
                        mybir.AluOpType.mult, mybir.AluOpType.add)
                    rms = sm2.tile([P, TPC], f32, tag="rms2")
                    nc.scalar.activation(rms, ms2, AF.Sqrt)
                    rsc = sm2.tile([P, TPC], f32, tag="rsc2")
                    nc.vector.reciprocal(rsc, rms)
                    hT = big.tile([P, DCH, TPC], ffdt, tag=htag, bufs=bbufs)
                    for k in range(DCH):
                        nc.vector.tensor_mul(hT[:, k, :], x2T[:, k, :], rsc)

                    # FFN in quarters of the intermediate dim
                    z = big.tile([P, DCH, TPC], f32, tag="big", bufs=bbufs)
                    for q in range(FQ):
                        u = big.tile([P, FPQ, TPC], ffdt, tag=htag, bufs=bbufs)
                        for fq in range(FPQ):
                            f = q * FPQ + fq
                            w1_sb = wstream.tile([P, DCH, P], ffdt, tag="w1b", bufs=3)
                            nc.sync.dma_start(w1_sb, w1_d.ap()[f])
                            psu = ps2.tile([P, TPC], f32, tag="ps2w", bufs=3)
                            for kc in range(DCH):
                                nc.tensor.matmul(psu, w1_sb[:, kc, :],
                                                 hT[:, kc, :],
                                                 start=(kc == 0),
                                                 stop=(kc == DCH - 1))
                            nc.scalar.activation(u[:, fq, :], psu, AF.Gelu)
                        for m in range(DCH):
                            w2_sb = w2stream.tile([P, FPQ, P], ffdt, tag="w2")
                            nc.sync.dma_start(w2_sb, w2_d.ap()[q, m])
                            psz = ps2.tile([P, TPC], f32, tag="psz", bufs=2)
                            for fq in range(FPQ):
                                nc.tensor.matmul(psz, w2_sb[:, fq, :],
                                                 u[:, fq, :],
                                                 start=(fq == 0),
                                                 stop=(fq == FPQ - 1))
                            if q == 0:
                                nc.vector.tensor_copy(z[:, m, :], psz)
                            else:
                                nc.vector.tensor_add(z[:, m, :], z[:, m, :], psz)

                    # final residual and store (transposed; host un-transposes)
                    x2r = big.tile([P, DCH, TPC], f32, tag="big", bufs=bbufs)
                    nc.sync.dma_start(x2r, x2_d.ap())
                    for m in range(DCH):
                        nc.vector.tensor_add(z[:, m, :], z[:, m, :], x2r[:, m, :])
                    nc.sync.dma_start(out_r, z)

    nc.compile()
    return nc


def _host_prep(x, attn_norm_w, wq, wk, wv, wo, ff_norm_w, w1, w2):
    f32 = np.float32
    xf = np.ascontiguousarray(x.reshape(B * T, DIM).T, dtype=f32)  # [D, BT]

    wq_e = (wq * attn_norm_w[None, :]).astype(f32)
    wk_e = (wk * attn_norm_w[None, :]).astype(f32)
    wv_e = (wv * attn_norm_w[None, :]).astype(f32)
    w1_e = (w1 * ff_norm_w[None, :]).astype(f32)

    wo_s = np.ascontiguousarray(
        wo.T.reshape(DCH, P, DCH, P).transpose(2, 1, 0, 3), dtype=f32)
    ffnp = ml_dtypes.bfloat16 if FFN_BF16 else f32
    w1_s = np.ascontiguousarray(
        w1_e.T.reshape(DCH, P, FCH, P).transpose(2, 1, 0, 3)).astype(ffnp)
    w2_s = np.ascontiguousarray(
        w2.T.reshape(FQ, FPQ, P, DCH, P).transpose(0, 3, 2, 1, 4)).astype(ffnp)

    rel = np.arange(QB // P)[:, None, None] * P + np.arange(P)[None, :, None]
    masks = (rel <= np.arange(QB)[None, None, :]).astype(f32)

    in_maps = []
    for c in range(NCORES):
        sl = slice(c * HDC, (c + 1) * HDC)
        xres = np.ascontiguousarray(np.concatenate(
            [xf[:, c * ASH:(c + 1) * ASH],
             xf[:, T + c * ASH:T + (c + 1) * ASH]], axis=1))
        in_maps.append({
            "xT": xf,
            "xresT": xres,
            "wqT": np.ascontiguousarray(wq_e[sl, :].T),
            "wkT": np.ascontiguousarray(wk_e[sl, :].T),
            "wvT": np.ascontiguousarray(wv_e[sl, :].T),
            "wo_s": wo_s,
            "w1_s": w1_s,
            "w2_s": w2_s,
            "masks": masks,
        })
    return in_maps


def _assemble(results, dtype):
    out = np.empty((B, T, DIM), dtype=np.float32)
    for c in range(NCORES):
        o = results[c]["outT"]  # [DIM, TPC] transposed
        on = o.T  # [TPC, DIM]
        out[0, c * ASH:(c + 1) * ASH, :] = on[:ASH]
        out[1, c * ASH:(c + 1) * ASH, :] = on[ASH:]
    return out.astype(dtype, copy=False)


def kernel(x, attn_norm_w, wq, wk, wv, wo, ff_norm_w, w1, w2):
    from concourse.bass_utils import run_bass_kernel_spmd

    x = np.asarray(x)
    if "nc" not in _CACHE:
        _CACHE["nc"] = _build_program()
    nc = _CACHE["nc"]

    in_maps = _host_prep(np.asarray(x, dtype=np.float32),
                         np.asarray(attn_norm_w), np.asarray(wq),
                         np.asarray(wk), np.asarray(wv), np.asarray(wo),
                         np.asarray(ff_norm_w), np.asarray(w1),
                         np.asarray(w2))
    res = run_bass_kernel_spmd(nc, in_maps, core_ids=list(range(NCORES)))
    return _assemble(res.results, x.dtype)

